# revision 59
# baseline (speedup 1.0000x reference)
"""Causal self-attention (B=4, T=2048, C=1024, H=16) on 8 trn2 NeuronCores.

Sharding: data-parallel over batch (4) x tensor-parallel over heads (2 groups
of 8).  Core c handles batch c//2, head group c%2.  Each core computes
qkv projection for its heads, causal flash-style attention, and a partial
output projection (over its 512 rows of w_proj).  The host sums the two
TP partials per batch and adds the bias.

All matmuls run in bf16 (inputs cast host-side; PSUM accumulation stays
fp32), 4x the PE rate of fp32 and well inside the 2e-2 gate.

The kernel is a single software-pipelined stream over 4 query chunks of
512 tokens.  QKV generation is just-in-time at 512-token-block
granularity: only block 0 is computed up front; each later block's QKV
matmul groups -- and each finished chunk's output-projection tiles -- are
"filler units" interleaved one head-pair at a time into the attention
stream, where they fill the PE bubbles left by the ACT-bound softmax.
w_qkv stays SBUF-resident so filler units issue no DMAs (the SP sequencer
FIFO would otherwise head-of-line block them behind stalled transposes).

Device layout notes:
  - host feeds x^T (feature-major) so the contraction dim (C) lands on SBUF
    partitions for the QKV matmuls with no on-device transpose.
  - Q^T,K^T produced feature-on-partition ([64h+d -> (p,sub)]), V produced
    token-on-partition with a ones column per head.
  - S^T tiles ([t2,t1]) are computed per (head-pair, q-chunk) with the two
    heads row-tiled (K=64 each, array rows 0-63 / 64-127); softmax is
    exp-without-max (scores are ~N(0,1); max over 268M scores ~ 6.5, safe
    in fp32).  Causal masking is multiplicative AFTER the exp (the strict
    upper triangle of the 128-wide diagonal slab is zeroed on the DVE in
    cheap all-bf16 SBUF ops); fully masked columns are never computed.
  - P@V is transposed relative to the usual flash layout: the P^T tile is
    the STATIONARY operand (lhsT, M=128 q-tokens) and [V|1] streams through
    (N=65), so each (k-tile, head, q-tile) matmul costs only 65 PE rows and
    the output lands q-token-on-partition with the softmax denominator in
    column 64 -- per-partition, so normalization is one small reciprocal
    plus one broadcast multiply, no cross-partition traffic.
  - the normalized O ([t1, head, d]) is flipped to O^T ([64h+d, t1]) for
    the projection by zero-engine-cost 128x128 DMA xbar transposes.
  - output projection consumes O^T directly as lhsT (contraction = head dim
    on partitions); host pre-permutes w_proj rows to match the O^T layout.
"""

import sys

sys.path.insert(0, "/opt/trn_rl_repo")

from collections import deque

import ml_dtypes
import numpy as np

import concourse.bass as bass
import concourse.bacc as bacc
import concourse.mybir as mybir
import concourse.tile as tile
from concourse.bass_utils import run_bass_kernel_spmd

F32 = mybir.dt.float32
BF16 = mybir.dt.bfloat16
P = 128
B, T, C = 4, 2048, 1024
H, D = 16, 64
NCORES = 8
TP = 2               # head-parallel groups
HL = H // TP         # 8 heads per core
CW = HL * D          # 512 head-cols per core
KS = C // P          # 8 contraction subtiles
NT = T // P          # 16 token tiles
NB = 4               # 512-token blocks / q-chunks
SCALE = float(1.0 / np.sqrt(D))

_CACHE = {}


def _build_module(dbg=False):
    nc = bacc.Bacc("TRN2", target_bir_lowering=False, debug=False,
                   num_devices=NCORES)
    xT = nc.dram_tensor("xT", (P, KS, T), BF16, kind="ExternalInput").ap()
    wqk = nc.dram_tensor("wqk", (8, P, KS, P), BF16, kind="ExternalInput").ap()
    wv = nc.dram_tensor("wv", (P, KS, CW), BF16, kind="ExternalInput").ap()
    wp = nc.dram_tensor("wp", (P, 4, C), BF16, kind="ExternalInput").ap()
    msk = nc.dram_tensor("msk", (P, P), BF16, kind="ExternalInput").ap()
    idn = nc.dram_tensor("idn", (P, P), BF16, kind="ExternalInput").ap()
    y = nc.dram_tensor("y", (NT, P, C), F32, kind="ExternalOutput").ap()
    if dbg:
        dk = nc.dram_tensor("dk", (P, 4, T), BF16, kind="ExternalOutput").ap()
        dv = nc.dram_tensor("dv", (P, NT, HL, 65), BF16, kind="ExternalOutput").ap()
        do = nc.dram_tensor("do", (P, 4, 4, 2, 64), BF16, kind="ExternalOutput").ap()
        doT = nc.dram_tensor("doT", (P, 4, T), BF16, kind="ExternalOutput").ap()
        dq = nc.dram_tensor("dq", (P, 4, 4, 512), BF16, kind="ExternalOutput").ap()

    Exp = mybir.ActivationFunctionType.Exp
    Mult = mybir.AluOpType.mult

    with tile.TileContext(nc) as tc, \
         tc.tile_pool(name="per", bufs=1) as per, \
         tc.tile_pool(name="strm", bufs=2) as strm, \
         tc.tile_pool(name="pp", bufs=2) as pp, \
         tc.tile_pool(name="pss", bufs=2, space="PSUM") as pss, \
         tc.tile_pool(name="psv", bufs=2, space="PSUM") as psv, \
         tc.tile_pool(name="psacc", bufs=1, space="PSUM") as psacc:

        # K^T rows r=64h+d live at (partition r%128, subtile r//128)
        k_sb = per.tile([P, 4, T], BF16)
        # V: [t2 partition, t-tile, head, 65]; cols 0-63 = V, col 64 = ones
        v_sb = per.tile([P, NT, HL, 65], BF16)
        # O, q-token-on-partition: [t1, qt-in-chunk, head-pair, head, d]
        o_sb = per.tile([P, 4, 4, 2, 64], BF16)
        oT_sb = per.tile([P, 4, T], BF16)
        mask_sb = per.tile([P, P], BF16)
        id_sb = per.tile([P, P], BF16)
        wqk_sb = per.tile([P, 8, KS, P], BF16)
        wv_sb = per.tile([P, KS, CW], BF16, tag="wbig")
        wp_sb = per.tile([P, 4, C], BF16, tag="wbig2")

        # preload the Exp activation table while the first DMAs run
        warm = per.tile([1, 2], F32)
        nc.vector.memset(warm, 0.0)
        nc.scalar.activation(warm, warm, Exp)

        xts = {}    # block -> xt tile

        def fetch_x(blk, split_first=False):
            t0 = blk * 512
            xt_t = strm.tile([P, KS, 512], BF16, tag="xt", bufs=2,
                             name=f"xt{blk}")
            xts[blk] = xt_t
            if split_first:
                nc.sync.dma_start(xt_t[:, 0, :], xT[:, 0, t0:t0 + 512])
            else:
                nc.sync.dma_start(xt_t, xT[:, :, t0:t0 + 512])

        def emit_qk(blk, mt):
            """One QKV-projection group: 512 tokens x 128 features."""
            q_t = q_sbs[blk]
            ps = psv.tile([P, CW], F32, tag="vf", name="ps_a")
            for ks in range(KS):
                nc.tensor.matmul(
                    ps,
                    lhsT=wqk_sb[:, mt, ks, :],
                    rhs=xts[blk][:, ks, :],
                    start=(ks == 0), stop=(ks == KS - 1))
            if mt < 4:
                nc.vector.tensor_copy(out=q_t[:, mt, :], in_=ps)
            else:
                nc.vector.tensor_copy(
                    out=k_sb[:, mt - 4, blk * 512:(blk + 1) * 512],
                    in_=ps)

        def emit_v(blk, tq):
            """V (+ones) for one 128-token tile."""
            tt = 4 * blk + tq
            ps_v = psv.tile([P, CW], F32, tag="vf", name="ps_v")
            for ks in range(KS):
                nc.tensor.matmul(
                    ps_v,
                    lhsT=xts[blk][:, ks, tq * 128:(tq + 1) * 128],
                    rhs=wv_sb[:, ks, :],
                    start=(ks == 0), stop=(ks == KS - 1))
            nc.vector.tensor_copy(out=v_sb[:, tt, :, 0:64],
                                  in_=ps_v.rearrange("p (h d) -> p h d", h=HL))

        def emit_proj(mt):
            """One output-projection tile: y[mt] = O^T[:, :, mt].T @ wp.
            nn-major so each 512-col half gets copied/stored while the
            other half's matmuls still run."""
            for nn in range(2):
                ps_y = psv.tile([P, CW], F32, tag="vf", name="ps_y")
                for jo in range(4):
                    nc.tensor.matmul(
                        ps_y,
                        lhsT=oT_sb[:, jo, mt * 128:(mt + 1) * 128],
                        rhs=wp_sb[:, jo, nn * 512:(nn + 1) * 512],
                        start=(jo == 0), stop=(jo == 3))
                if mt == NT - 1:
                    # drain-critical last tile: quarter-size copies on
                    # alternating engines so stores start asap
                    for qq in range(2):
                        y_sb = pp.tile([P, 256], F32, tag="yq", bufs=4)
                        if qq == 0:
                            nc.vector.tensor_copy(
                                out=y_sb, in_=ps_y[:, 0:256])
                        else:
                            nc.scalar.copy(y_sb, ps_y[:, 256:512])
                        nc.sync.dma_start(
                            y[mt][:, nn * 512 + qq * 256:
                                  nn * 512 + (qq + 1) * 256], y_sb)
                else:
                    y_sb = pp.tile([P, 512], F32, tag="y", bufs=3)
                    nc.vector.tensor_copy(out=y_sb, in_=ps_y)
                    nc.sync.dma_start(y[mt][:, nn * 512:(nn + 1) * 512], y_sb)

        # ---- prologue: startup-critical DMAs, then block 0's QKV ----
        fetch_x(0, split_first=True)
        nc.sync.dma_start(wqk_sb[:, 4, 0:2], wqk[4][:, 0:2])
        nc.sync.dma_start(xts[0][:, 1:3, :], xT[:, 1:3, 0:512])
        nc.sync.dma_start(wqk_sb[:, 4, 2:], wqk[4][:, 2:])
        nc.sync.dma_start(xts[0][:, 3:, :], xT[:, 3:, 0:512])
        for mt in (5, 6, 7, 0, 1, 2, 3):
            nc.sync.dma_start(wqk_sb[:, mt], wqk[mt])
        nc.sync.dma_start(mask_sb, msk)
        nc.sync.dma_start(id_sb, idn)
        nc.sync.dma_start(wv_sb, wv)
        nc.sync.dma_start(wp_sb, wp)
        nc.vector.memset(v_sb[:, :, :, 64:65], 1.0)

        q_sbs = {0: strm.tile([P, 4, 512], BF16, tag="q", bufs=2, name="q0")}

        for mt in (4, 5, 6, 7, 0):
            emit_qk(0, mt)
        for tq in range(4):
            emit_v(0, tq)

        # filler units consumed inside the attention stream, paced so each
        # head-pair gets an even share of the chunk's unit budget
        filler = deque()
        consumed = [0]

        def hook(n):
            for _ in range(min(n, len(filler))):
                filler.popleft()()
                consumed[0] += 1

        # ---- the pipelined attention stream over 4 q-chunks ----
        for c in range(NB):
            ntile = 4 * c + 4
            # next block: x fetch + QKV units; finished chunks: proj units
            if c + 1 < NB:
                fetch_x(c + 1)
                q_sbs[c + 1] = strm.tile([P, 4, 512], BF16, tag="q", bufs=2,
                                         name=f"q{c + 1}")
                if c == 0:
                    filler.extend([
                        (lambda mt=mt: emit_qk(0, mt)) for mt in (1, 2, 3)])
                for mt in (4, 5, 6, 7, 0, 1, 2, 3):
                    filler.append(lambda b=c + 1, mt=mt: emit_qk(b, mt))
                for tq in range(4):
                    filler.append(lambda b=c + 1, tq=tq: emit_v(b, tq))
            else:
                for mt in range(12):
                    filler.append(lambda mt=mt: emit_proj(mt))
            chunk_units = len(filler)
            consumed[0] = 0
            shr = (4, 8, 12, 16) if c + 1 < NB else (3, 7, 11, 16)

            for pr in range(4):
                h0 = 2 * pr
                # Streaming P@V with one active accumulation group per psum
                # bank (a start=True clears the whole bank's has_written, so
                # concurrent groups in one bank corrupt each other).  Bank 0
                # serves q-tiles 0,2; bank 1 serves 1,3.  Each (qt, head)
                # group streams tile contributions as the exps land; its
                # successor catches up with a burst (all p_t tiles are kept
                # for the whole head-pair).
                p_list = []
                accs = {}
                bank_groups = ([(0, 0), (0, 1), (2, 0), (2, 1)],
                               [(1, 0), (1, 1), (3, 0), (3, 1)])
                bank_state = [{"gi": 0, "nt": 0}, {"gi": 0, "nt": 0}]

                def pv_mm(qt, hh, tt):
                    jt = 4 * c + qt
                    nc.tensor.matmul(
                        accs[qt][:, hh, :],
                        lhsT=p_list[tt][:, hh, qt * 128:(qt + 1) * 128],
                        rhs=v_sb[:, tt, h0 + hh, 0:65],
                        start=(tt == 0), stop=(tt == jt),
                        skip_group_check=True)

                def norm_qt(qt):
                    # denominator sits per-partition in psum column 64:
                    # tiny reciprocal + one broadcast multiply
                    acc = accs[qt]
                    rd = pp.tile([P, 2], F32, tag="rd", bufs=4)
                    nc.vector.reciprocal(rd, acc[:, :, 64])
                    nc.vector.tensor_tensor(
                        out=o_sb[:, qt, pr, :, :], in0=acc[:, :, 0:64],
                        in1=rd[:, :, None].to_broadcast((P, 2, 64)),
                        op=Mult)

                def advance(bank, tt_now):
                    st = bank_state[bank]
                    groups = bank_groups[bank]
                    while st["gi"] < len(groups):
                        qt, hh = groups[st["gi"]]
                        jt = 4 * c + qt
                        if hh == 0 and st["nt"] == 0:
                            accs[qt] = psacc.tile(
                                [P, 2, 65], F32, tag=f"acc{bank}",
                                name=f"acc{qt}")
                        hi = min(jt, tt_now)
                        for tt in range(st["nt"], hi + 1):
                            pv_mm(qt, hh, tt)
                        if hi == jt:
                            st["gi"] += 1
                            st["nt"] = 0
                            if hh == 1:
                                norm_qt(qt)
                                # let the next S/exp slip in before the
                                # successor's start (it waits on this norm's
                                # psum reads -- avoid a PE FIFO stall)
                                if tt_now < ntile - 1:
                                    break
                            continue
                        st["nt"] = hi + 1
                        break

                for tt in range(ntile):
                    i = tt - 4 * c  # diagonal index (>=0 on diagonal)
                    col0 = 128 * i if i >= 0 else 0
                    s_ps = pss.tile([P, 2, 512], F32, tag="s")
                    for hh, pb in ((0, 0), (1, 64)):
                        nc.tensor.matmul(
                            s_ps[:, hh, col0:512],
                            lhsT=k_sb[pb:pb + 64, pr, tt * 128:(tt + 1) * 128],
                            rhs=q_sbs[c][pb:pb + 64, pr, col0:512],
                            start=True, stop=True)
                    p_t = pp.tile([P, 2, 512], BF16, tag="p", bufs=17)
                    p_list.append(p_t)
                    nc.scalar.activation(
                        p_t[:, :, col0:512], s_ps[:, :, col0:512],
                        Exp, scale=SCALE)
                    if i >= 0:
                        # zero the strict upper triangle of the diagonal
                        # 128-slab (post-exp multiplicative causal mask;
                        # all-bf16 SBUF op -> DVE fast path)
                        nc.vector.tensor_tensor(
                            out=p_t[:, :, col0:col0 + 128],
                            in0=p_t[:, :, col0:col0 + 128],
                            in1=mask_sb[:, None, :].to_broadcast((P, 2, P)),
                            op=Mult)
                    advance(0, tt)
                    advance(1, tt)
                    if tt % 3 == 2 and \
                            consumed[0] < chunk_units * (pr * ntile + tt + 1) \
                            // (4 * ntile):
                        # drip one filler unit into the stream
                        hook(1)
                # top up to this head-pair's proportional share
                hook(chunk_units * shr[pr] // 16 - consumed[0])

                # flip to O^T for the projection: 4 PE-mode transposes into
                # one psum tile + one DVE copy out.  Deferred via the filler
                # queue so the PE FIFO never stalls waiting on the norm.
                def emit_transpose(c=c, pr=pr):
                    ps_t = psv.tile([P, 2, CW], BF16, tag="vf", name="ps_t")
                    for qt in range(4):
                        nc.tensor.transpose(
                            ps_t[:, 0, qt * 128:(qt + 1) * 128],
                            o_sb[:, qt, pr, :, :], id_sb)
                    nc.vector.tensor_copy(
                        out=oT_sb[:, pr, c * 512:(c + 1) * 512],
                        in_=ps_t[:, 0, :])
                filler.append(emit_transpose)

        # drain any leftover filler, then the last chunk's projection
        hook(len(filler))
        for mt in range(12, NT):
            emit_proj(mt)
        if dbg:
            nc.sync.dma_start(dk, k_sb)
            nc.sync.dma_start(dv, v_sb)
            nc.sync.dma_start(do, o_sb)
            nc.sync.dma_start(doT, oT_sb)
            for cc2 in range(4):
                if cc2 in q_sbs:
                    nc.sync.dma_start(dq[:, cc2], q_sbs[cc2])

    nc.compile()
    return nc


def get_module(dbg=False):
    key = ("dbg" if dbg else "nc")
    if key not in _CACHE:
        _CACHE[key] = _build_module(dbg)
    return _CACHE[key]


def _wp_perm():
    # O^T row layout: (partition p, subtile jo) <-> head h = 2*jo + (p>=64),
    # dim d = p % 64; w_proj row (within this core's 512) = 64*h + d.
    p = np.arange(P)[:, None]
    jo = np.arange(4)[None, :]
    h = 2 * jo + (p >= 64)
    return (64 * h + p % 64).reshape(-1)


def make_core_inputs(x, w_qkv, w_proj, core):
    b, g = core // TP, core % TP
    xt = np.ascontiguousarray(x[b].T)                    # [C, T]
    xt = np.ascontiguousarray(xt.reshape(KS, P, T).transpose(1, 0, 2))
    qcols = w_qkv[:, g * CW:(g + 1) * CW]
    kcols = w_qkv[:, C + g * CW:C + (g + 1) * CW]
    wqk = np.concatenate([qcols, kcols], axis=1)         # [C, 1024]
    wqk = np.ascontiguousarray(
        wqk.reshape(KS, P, 8, P).transpose(2, 1, 0, 3))  # [mt, p, ko, m]
    wv = w_qkv[:, 2 * C + g * CW:2 * C + (g + 1) * CW]
    wv = np.ascontiguousarray(wv.reshape(KS, P, CW).transpose(1, 0, 2))
    wp = np.ascontiguousarray(
        w_proj[g * CW:(g + 1) * CW, :][_wp_perm()].reshape(P, 4, C))
    # multiplicative causal mask for the diagonal slab: keep k <= q
    mask = (np.arange(P)[:, None] <= np.arange(P)[None, :])
    bf16 = ml_dtypes.bfloat16
    return {"xT": xt.astype(bf16), "wqk": wqk.astype(bf16),
            "wv": wv.astype(bf16), "wp": wp.astype(bf16),
            "msk": np.ascontiguousarray(mask.astype(np.float32)).astype(bf16),
            "idn": np.eye(P, dtype=np.float32).astype(bf16)}


def _run(inputs, trace=False):
    x = np.asarray(inputs["x"], np.float32)
    w_qkv = np.asarray(inputs["w_qkv"], np.float32)
    w_proj = np.asarray(inputs["w_proj"], np.float32)
    b_proj = np.asarray(inputs["b_proj"], np.float32)
    nc = get_module()
    in_maps = [make_core_inputs(x, w_qkv, w_proj, core)
               for core in range(NCORES)]
    res = run_bass_kernel_spmd(nc, in_maps, core_ids=list(range(NCORES)),
                               trace=trace)
    outs = [np.asarray(r["y"], np.float32).reshape(T, C) for r in res.results]
    yfull = np.empty((B, T, C), np.float32)
    for b in range(B):
        yfull[b] = outs[TP * b] + outs[TP * b + 1] + b_proj[None, :]
    return yfull, res


def kernel(**inputs):
    y, _ = _run(inputs, trace=False)
    return y


# revision 60
# speedup vs baseline: 1.1927x; 1.1927x over previous
"""Causal self-attention (B=4, T=2048, C=1024, H=16) on 8 trn2 NeuronCores.

Sharding: data-parallel over batch (4) x tensor-parallel over heads (2 groups
of 8).  Core c handles batch c//2, head group c%2.  Each core computes
qkv projection for its heads, causal flash-style attention, and a partial
output projection (over its 512 rows of w_proj).  The host sums the two
TP partials per batch and adds the bias.

All matmuls run in bf16 (inputs cast host-side; PSUM accumulation stays
fp32), 4x the PE rate of fp32 and well inside the 2e-2 gate.

The kernel is a single software-pipelined stream over 4 query chunks of
512 tokens.  QKV generation is just-in-time at 512-token-block
granularity: only block 0 is computed up front; each later block's QKV
matmul groups -- and each finished chunk's output-projection tiles -- are
"filler units" interleaved one head-pair at a time into the attention
stream, where they fill the PE bubbles left by the ACT-bound softmax.
w_qkv stays SBUF-resident so filler units issue no DMAs (the SP sequencer
FIFO would otherwise head-of-line block them behind stalled transposes).

Device layout notes:
  - host feeds x^T (feature-major) so the contraction dim (C) lands on SBUF
    partitions for the QKV matmuls with no on-device transpose.
  - Q^T,K^T produced feature-on-partition ([64h+d -> (p,sub)]), V produced
    token-on-partition with a ones column per head.
  - S^T tiles ([t2,t1]) are computed per (head-pair, q-chunk) with the two
    heads row-tiled (K=64 each, array rows 0-63 / 64-127); softmax is
    exp-without-max (scores are ~N(0,1); max over 268M scores ~ 6.5, safe
    in fp32).  Causal masking is multiplicative AFTER the exp (the strict
    upper triangle of the 128-wide diagonal slab is zeroed on the DVE in
    cheap all-bf16 SBUF ops); fully masked columns are never computed.
  - P@V is transposed relative to the usual flash layout: the P^T tile is
    the STATIONARY operand (lhsT, M=128 q-tokens) and [V|1] streams through
    (N=65), so each (k-tile, head, q-tile) matmul costs only 65 PE rows and
    the output lands q-token-on-partition with the softmax denominator in
    column 64 -- per-partition, so normalization is one small reciprocal
    plus one broadcast multiply, no cross-partition traffic.
  - the normalized O ([t1, head, d]) is flipped to O^T ([64h+d, t1]) for
    the projection by zero-engine-cost 128x128 DMA xbar transposes.
  - output projection consumes O^T directly as lhsT (contraction = head dim
    on partitions); host pre-permutes w_proj rows to match the O^T layout.
"""

import sys

sys.path.insert(0, "/opt/trn_rl_repo")

from collections import deque

import ml_dtypes
import numpy as np

import concourse.bass as bass
import concourse.bacc as bacc
import concourse.mybir as mybir
import concourse.tile as tile
from concourse.bass_utils import run_bass_kernel_spmd

F32 = mybir.dt.float32
BF16 = mybir.dt.bfloat16
P = 128
B, T, C = 4, 2048, 1024
H, D = 16, 64
NCORES = 8
TP = 2               # head-parallel groups
HL = H // TP         # 8 heads per core
CW = HL * D          # 512 head-cols per core
KS = C // P          # 8 contraction subtiles
NT = T // P          # 16 token tiles
NB = 4               # 512-token blocks / q-chunks
SCALE = float(1.0 / np.sqrt(D))

_CACHE = {}


def _build_module(dbg=False):
    nc = bacc.Bacc("TRN2", target_bir_lowering=False, debug=False,
                   num_devices=NCORES)
    xT = nc.dram_tensor("xT", (P, KS, T), BF16, kind="ExternalInput").ap()
    wqk = nc.dram_tensor("wqk", (8, P, KS, P), BF16, kind="ExternalInput").ap()
    wv = nc.dram_tensor("wv", (P, KS, CW), BF16, kind="ExternalInput").ap()
    wp = nc.dram_tensor("wp", (P, 4, C), BF16, kind="ExternalInput").ap()
    msk = nc.dram_tensor("msk", (P, P), BF16, kind="ExternalInput").ap()
    idn = nc.dram_tensor("idn", (P, P), BF16, kind="ExternalInput").ap()
    y = nc.dram_tensor("y", (NT, P, C), F32, kind="ExternalOutput").ap()
    if dbg:
        dk = nc.dram_tensor("dk", (P, 4, T), BF16, kind="ExternalOutput").ap()
        dv = nc.dram_tensor("dv", (P, NT, HL, 65), BF16, kind="ExternalOutput").ap()
        do = nc.dram_tensor("do", (P, 4, 4, 2, 64), BF16, kind="ExternalOutput").ap()
        doT = nc.dram_tensor("doT", (P, 4, T), BF16, kind="ExternalOutput").ap()
        dq = nc.dram_tensor("dq", (P, 4, 4, 512), BF16, kind="ExternalOutput").ap()

    Exp = mybir.ActivationFunctionType.Exp
    Mult = mybir.AluOpType.mult

    with tile.TileContext(nc) as tc, \
         tc.tile_pool(name="per", bufs=1) as per, \
         tc.tile_pool(name="strm", bufs=2) as strm, \
         tc.tile_pool(name="pp", bufs=2) as pp, \
         tc.tile_pool(name="pss", bufs=2, space="PSUM") as pss, \
         tc.tile_pool(name="psv", bufs=2, space="PSUM") as psv, \
         tc.tile_pool(name="psacc", bufs=1, space="PSUM") as psacc:

        # K^T rows r=64h+d live at (partition r%128, subtile r//128)
        k_sb = per.tile([P, 4, T], BF16)
        # V: [t2 partition, t-tile, head, 65]; cols 0-63 = V, col 64 = ones
        v_sb = per.tile([P, NT, HL, 65], BF16)
        # O, q-token-on-partition: [t1, qt-in-chunk, head-pair, head, d]
        o_sb = per.tile([P, 4, 4, 2, 64], BF16)
        oT_sb = per.tile([P, 4, T], BF16)
        mask_sb = per.tile([P, P], BF16)
        id_sb = per.tile([P, P], BF16)
        wqk_sb = per.tile([P, 8, KS, P], BF16)
        wv_sb = per.tile([P, KS, CW], BF16, tag="wbig")
        wp_sb = per.tile([P, 4, C], BF16, tag="wbig2")

        # preload the Exp activation table while the first DMAs run
        warm = per.tile([1, 2], F32)
        nc.vector.memset(warm, 0.0)
        nc.scalar.activation(warm, warm, Exp)

        xts = {}    # block -> xt tile

        def fetch_x(blk, split_first=False):
            t0 = blk * 512
            xt_t = strm.tile([P, KS, 512], BF16, tag="xt", bufs=2,
                             name=f"xt{blk}")
            xts[blk] = xt_t
            if split_first:
                nc.sync.dma_start(xt_t[:, 0, :], xT[:, 0, t0:t0 + 512])
            else:
                nc.sync.dma_start(xt_t, xT[:, :, t0:t0 + 512])

        def emit_qk(blk, mt):
            """One QKV-projection group: 512 tokens x 128 features."""
            q_t = q_sbs[blk]
            ps = psv.tile([P, CW], F32, tag="vf", name="ps_a")
            for ks in range(KS):
                nc.tensor.matmul(
                    ps,
                    lhsT=wqk_sb[:, mt, ks, :],
                    rhs=xts[blk][:, ks, :],
                    start=(ks == 0), stop=(ks == KS - 1))
            if mt < 4:
                nc.vector.tensor_copy(out=q_t[:, mt, :], in_=ps)
            else:
                nc.vector.tensor_copy(
                    out=k_sb[:, mt - 4, blk * 512:(blk + 1) * 512],
                    in_=ps)

        def emit_v(blk, tq):
            """V (+ones) for one 128-token tile."""
            tt = 4 * blk + tq
            ps_v = psv.tile([P, CW], F32, tag="vf", name="ps_v")
            for ks in range(KS):
                nc.tensor.matmul(
                    ps_v,
                    lhsT=xts[blk][:, ks, tq * 128:(tq + 1) * 128],
                    rhs=wv_sb[:, ks, :],
                    start=(ks == 0), stop=(ks == KS - 1))
            nc.vector.tensor_copy(out=v_sb[:, tt, :, 0:64],
                                  in_=ps_v.rearrange("p (h d) -> p h d", h=HL))

        def emit_proj_nn(mt, nn):
            """Half an output-projection tile (one 512-col half of
            y[mt] = O^T[:, :, mt].T @ wp)."""
            if True:
                ps_y = psv.tile([P, CW], F32, tag="vf", name="ps_y")
                for jo in range(4):
                    nc.tensor.matmul(
                        ps_y,
                        lhsT=oT_sb[:, jo, mt * 128:(mt + 1) * 128],
                        rhs=wp_sb[:, jo, nn * 512:(nn + 1) * 512],
                        start=(jo == 0), stop=(jo == 3))
                if mt == NT - 1:
                    # drain-critical last tile: quarter-size copies on
                    # alternating engines so stores start asap
                    for qq in range(2):
                        y_sb = pp.tile([P, 256], F32, tag="yq", bufs=4)
                        if qq == 0:
                            nc.vector.tensor_copy(
                                out=y_sb, in_=ps_y[:, 0:256])
                        else:
                            nc.scalar.copy(y_sb, ps_y[:, 256:512])
                        nc.sync.dma_start(
                            y[mt][:, nn * 512 + qq * 256:
                                  nn * 512 + (qq + 1) * 256], y_sb)
                else:
                    y_sb = pp.tile([P, 512], F32, tag="y", bufs=3)
                    nc.vector.tensor_copy(out=y_sb, in_=ps_y)
                    nc.sync.dma_start(y[mt][:, nn * 512:(nn + 1) * 512], y_sb)

        # ---- prologue: startup-critical DMAs, then block 0's QKV ----
        fetch_x(0, split_first=True)
        nc.sync.dma_start(wqk_sb[:, 4, 0:2], wqk[4][:, 0:2])
        nc.sync.dma_start(xts[0][:, 1:3, :], xT[:, 1:3, 0:512])
        nc.sync.dma_start(wqk_sb[:, 4, 2:], wqk[4][:, 2:])
        nc.sync.dma_start(xts[0][:, 3:, :], xT[:, 3:, 0:512])
        for mt in (5, 6, 7, 0, 1, 2, 3):
            nc.sync.dma_start(wqk_sb[:, mt], wqk[mt])
        nc.sync.dma_start(mask_sb, msk)
        nc.sync.dma_start(id_sb, idn)
        nc.sync.dma_start(wv_sb, wv)
        nc.sync.dma_start(wp_sb, wp)
        nc.vector.memset(v_sb[:, :, :, 64:65], 1.0)

        q_sbs = {0: strm.tile([P, 4, 512], BF16, tag="q", bufs=2, name="q0")}

        for mt in (4, 5, 6, 7, 0):
            emit_qk(0, mt)
        for tq in range(4):
            emit_v(0, tq)

        # filler units consumed inside the attention stream, paced so each
        # head-pair gets an even share of the chunk's unit budget
        filler = deque()
        consumed = [0]

        def hook(n):
            for _ in range(min(n, len(filler))):
                filler.popleft()()
                consumed[0] += 1

        # ---- the pipelined attention stream over 4 q-chunks ----
        for c in range(NB):
            ntile = 4 * c + 4
            # next block: x fetch + QKV units; finished chunks: proj units
            if c + 1 < NB:
                fetch_x(c + 1)
                q_sbs[c + 1] = strm.tile([P, 4, 512], BF16, tag="q", bufs=2,
                                         name=f"q{c + 1}")
                if c == 0:
                    filler.extend([
                        (lambda mt=mt: emit_qk(0, mt)) for mt in (1, 2, 3)])
                for mt in (4, 5, 6, 7, 0, 1, 2, 3):
                    filler.append(lambda b=c + 1, mt=mt: emit_qk(b, mt))
                for tq in range(4):
                    filler.append(lambda b=c + 1, tq=tq: emit_v(b, tq))
            else:
                for mt in range(12):
                    for nn in range(2):
                        filler.append(
                            lambda mt=mt, nn=nn: emit_proj_nn(mt, nn))
            chunk_units = len(filler)
            consumed[0] = 0
            shr = (4, 8, 12, 16) if c + 1 < NB else (3, 7, 11, 16)

            for pr in range(4):
                h0 = 2 * pr
                # Streaming P@V with one active accumulation group per psum
                # bank (a start=True clears the whole bank's has_written, so
                # concurrent groups in one bank corrupt each other).  Bank 0
                # serves q-tiles 0,2; bank 1 serves 1,3.  Each (qt, head)
                # group streams tile contributions as the exps land; its
                # successor catches up with a burst (all p_t tiles are kept
                # for the whole head-pair).
                p_list = []
                accs = {}
                bank_groups = ([(0, 0), (0, 1), (2, 0), (2, 1)],
                               [(1, 0), (1, 1), (3, 0), (3, 1)])
                bank_state = [{"gi": 0, "nt": 0}, {"gi": 0, "nt": 0}]

                def pv_mm(qt, hh, tt):
                    jt = 4 * c + qt
                    nc.tensor.matmul(
                        accs[qt][:, hh, :],
                        lhsT=p_list[tt][:, hh, qt * 128:(qt + 1) * 128],
                        rhs=v_sb[:, tt, h0 + hh, 0:65],
                        start=(tt == 0), stop=(tt == jt),
                        skip_group_check=True)

                def norm_qt(qt):
                    # denominator sits per-partition in psum column 64:
                    # tiny reciprocal + one broadcast multiply
                    acc = accs[qt]
                    rd = pp.tile([P, 2], F32, tag="rd", bufs=4)
                    nc.vector.reciprocal(rd, acc[:, :, 64])
                    nc.vector.tensor_tensor(
                        out=o_sb[:, qt, pr, :, :], in0=acc[:, :, 0:64],
                        in1=rd[:, :, None].to_broadcast((P, 2, 64)),
                        op=Mult)

                def advance(bank, tt_now):
                    st = bank_state[bank]
                    groups = bank_groups[bank]
                    while st["gi"] < len(groups):
                        qt, hh = groups[st["gi"]]
                        jt = 4 * c + qt
                        if hh == 0 and st["nt"] == 0:
                            accs[qt] = psacc.tile(
                                [P, 2, 65], F32, tag=f"acc{bank}",
                                name=f"acc{qt}")
                        hi = min(jt, tt_now)
                        for tt in range(st["nt"], hi + 1):
                            pv_mm(qt, hh, tt)
                        if hi == jt:
                            st["gi"] += 1
                            st["nt"] = 0
                            if hh == 1:
                                norm_qt(qt)
                                # let the next S/exp slip in before the
                                # successor's start (it waits on this norm's
                                # psum reads -- avoid a PE FIFO stall)
                                if tt_now < ntile - 1:
                                    break
                            continue
                        st["nt"] = hi + 1
                        break

                for tt in range(ntile):
                    i = tt - 4 * c  # diagonal index (>=0 on diagonal)
                    col0 = 128 * i if i >= 0 else 0
                    s_ps = pss.tile([P, 2, 512], F32, tag="s")
                    for hh, pb in ((0, 0), (1, 64)):
                        nc.tensor.matmul(
                            s_ps[:, hh, col0:512],
                            lhsT=k_sb[pb:pb + 64, pr, tt * 128:(tt + 1) * 128],
                            rhs=q_sbs[c][pb:pb + 64, pr, col0:512],
                            start=True, stop=True)
                    p_t = pp.tile([P, 2, 512], BF16, tag="p", bufs=17)
                    p_list.append(p_t)
                    nc.scalar.activation(
                        p_t[:, :, col0:512], s_ps[:, :, col0:512],
                        Exp, scale=SCALE)
                    if i >= 0:
                        # zero the strict upper triangle of the diagonal
                        # 128-slab (post-exp multiplicative causal mask;
                        # all-bf16 SBUF op -> DVE fast path)
                        nc.vector.tensor_tensor(
                            out=p_t[:, :, col0:col0 + 128],
                            in0=p_t[:, :, col0:col0 + 128],
                            in1=mask_sb[:, None, :].to_broadcast((P, 2, P)),
                            op=Mult)
                    advance(0, tt)
                    advance(1, tt)
                    if tt % 3 == 2 and \
                            consumed[0] < chunk_units * (pr * ntile + tt + 1) \
                            // (4 * ntile):
                        # drip one filler unit into the stream
                        hook(1)
                # top up to this head-pair's proportional share
                hook(chunk_units * shr[pr] // 16 - consumed[0])

                # flip to O^T for the projection: 4 PE-mode transposes into
                # one psum tile + one DVE copy out.  Deferred via the filler
                # queue so the PE FIFO never stalls waiting on the norm.
                def emit_transpose(c=c, pr=pr):
                    ps_t = psv.tile([P, 2, CW], BF16, tag="vf", name="ps_t")
                    for qt in range(4):
                        nc.tensor.transpose(
                            ps_t[:, 0, qt * 128:(qt + 1) * 128],
                            o_sb[:, qt, pr, :, :], id_sb)
                    nc.vector.tensor_copy(
                        out=oT_sb[:, pr, c * 512:(c + 1) * 512],
                        in_=ps_t[:, 0, :])
                filler.append(emit_transpose)

        # drain any leftover filler, then the last chunk's projection
        hook(len(filler))
        for mt in range(12, NT):
            for nn in range(2):
                emit_proj_nn(mt, nn)
        if dbg:
            nc.sync.dma_start(dk, k_sb)
            nc.sync.dma_start(dv, v_sb)
            nc.sync.dma_start(do, o_sb)
            nc.sync.dma_start(doT, oT_sb)
            for cc2 in range(4):
                if cc2 in q_sbs:
                    nc.sync.dma_start(dq[:, cc2], q_sbs[cc2])

    nc.compile()
    return nc


def get_module(dbg=False):
    key = ("dbg" if dbg else "nc")
    if key not in _CACHE:
        _CACHE[key] = _build_module(dbg)
    return _CACHE[key]


def _wp_perm():
    # O^T row layout: (partition p, subtile jo) <-> head h = 2*jo + (p>=64),
    # dim d = p % 64; w_proj row (within this core's 512) = 64*h + d.
    p = np.arange(P)[:, None]
    jo = np.arange(4)[None, :]
    h = 2 * jo + (p >= 64)
    return (64 * h + p % 64).reshape(-1)


def make_core_inputs(x, w_qkv, w_proj, core):
    b, g = core // TP, core % TP
    xt = np.ascontiguousarray(x[b].T)                    # [C, T]
    xt = np.ascontiguousarray(xt.reshape(KS, P, T).transpose(1, 0, 2))
    qcols = w_qkv[:, g * CW:(g + 1) * CW]
    kcols = w_qkv[:, C + g * CW:C + (g + 1) * CW]
    wqk = np.concatenate([qcols, kcols], axis=1)         # [C, 1024]
    wqk = np.ascontiguousarray(
        wqk.reshape(KS, P, 8, P).transpose(2, 1, 0, 3))  # [mt, p, ko, m]
    wv = w_qkv[:, 2 * C + g * CW:2 * C + (g + 1) * CW]
    wv = np.ascontiguousarray(wv.reshape(KS, P, CW).transpose(1, 0, 2))
    wp = np.ascontiguousarray(
        w_proj[g * CW:(g + 1) * CW, :][_wp_perm()].reshape(P, 4, C))
    # multiplicative causal mask for the diagonal slab: keep k <= q
    mask = (np.arange(P)[:, None] <= np.arange(P)[None, :])
    bf16 = ml_dtypes.bfloat16
    return {"xT": xt.astype(bf16), "wqk": wqk.astype(bf16),
            "wv": wv.astype(bf16), "wp": wp.astype(bf16),
            "msk": np.ascontiguousarray(mask.astype(np.float32)).astype(bf16),
            "idn": np.eye(P, dtype=np.float32).astype(bf16)}


def _run(inputs, trace=False):
    x = np.asarray(inputs["x"], np.float32)
    w_qkv = np.asarray(inputs["w_qkv"], np.float32)
    w_proj = np.asarray(inputs["w_proj"], np.float32)
    b_proj = np.asarray(inputs["b_proj"], np.float32)
    nc = get_module()
    in_maps = [make_core_inputs(x, w_qkv, w_proj, core)
               for core in range(NCORES)]
    res = run_bass_kernel_spmd(nc, in_maps, core_ids=list(range(NCORES)),
                               trace=trace)
    outs = [np.asarray(r["y"], np.float32).reshape(T, C) for r in res.results]
    yfull = np.empty((B, T, C), np.float32)
    for b in range(B):
        yfull[b] = outs[TP * b] + outs[TP * b + 1] + b_proj[None, :]
    return yfull, res


def kernel(**inputs):
    y, _ = _run(inputs, trace=False)
    return y
